# revision 1
# baseline (speedup 1.0000x reference)
"""Trainium2 Bass kernel for nn_Attention_62010737820049 (optimized v2).

Transformer-XL-style relative-position attention block + LN + FFN,
data-parallel over batch across 8 NeuronCores (4 batches per core, no
collectives). Improvements over v1:
  - k-projection folded: scores = (qu @ Wke) @ hf^T (saves 64 matmul
    units/batch vs materializing k over the W=1024 axis)
  - all PE transposes run on bf16 data with a bf16 identity (1 cyc/row
    instead of 2 for f32)
  - scores/b phases skip fully-masked 128-column chunks
  - biases are applied on DVE (broadcast rows) instead of f32r PE matmuls
  - att transposes are emitted after the val matmuls so the in-order PE
    queue doesn't stall waiting on softmax DVE work
The circulant rel-pos gather stays a strided DMA re-read of q@kr^T
through a DRAM scratch buffer.
"""

import os
import sys

sys.path.insert(0, "/opt/trn_rl_repo")

import numpy as np
import ml_dtypes

B, C, MEM, D = 32, 512, 512, 1024
W = C + MEM           # 1024
FF = 4 * D            # 4096
P = 128
NCORES = 8
BPC = int(os.environ.get("KERNEL_BPC", str(B // NCORES)))  # batches per core
CH_D, CH_C, CH_W, CH_F = D // P, C // P, W // P, FF // P   # 8, 4, 8, 32
EPS = 1e-5
ISQ = 1.0 / 32.0      # 1/sqrt(D)

_cached = {}


def _emit(nc, tc, tn):
    import concourse.bass as bass
    import concourse.mybir as mybir
    from concourse.masks import make_identity

    f32 = mybir.dt.float32
    bf16 = mybir.dt.bfloat16
    AF = mybir.ActivationFunctionType
    OP = mybir.AluOpType

    def vtt(out, a, b, op):
        return nc.vector.tensor_tensor(out=out, in0=a, in1=b, op=op)

    xs, hs, outs = tn["x"], tn["h"], tn["out"]

    with (
        tc.tile_pool(name="constp", bufs=1) as constp,
        tc.tile_pool(name="wp", bufs=12) as wp,          # [P,1024]bf16 stream
        tc.tile_pool(name="a1", bufs=6) as a1,           # [P,1024]f32 work
        tc.tile_pool(name="natb", bufs=8) as natbp,      # [P,1024]bf16
        tc.tile_pool(name="eatt", bufs=12) as eattp,     # [P,1024]bf16
        tc.tile_pool(name="w2p", bufs=3) as w2p,         # [P,1024]bf16
        tc.tile_pool(name="hfp", bufs=8) as hfp,         # [P,1024]bf16 (hf)
        tc.tile_pool(name="vp", bufs=8) as vp,           # [P,1024]bf16 (val)
        tc.tile_pool(name="a5", bufs=24) as a5,          # [P,512]bf16 small
        tc.tile_pool(name="psb5", bufs=2) as psb5,       # [P,512]f32 (b out)
        tc.tile_pool(name="w1p", bufs=4) as w1p,         # [P,8,128]bf16
        tc.tile_pool(name="ftp", bufs=32) as ftp,        # fT
        tc.tile_pool(name="stp", bufs=28) as stp,        # [P,1] stats
        tc.tile_pool(name="psp", bufs=8, space="PSUM") as psp,
        tc.tile_pool(name="pdram", bufs=2, space="DRAM") as pdram,
    ):
        identb = constp.tile([P, P], bf16, name="identb", tag="identb")
        make_identity(nc, identb[:])
        bq_sb = constp.tile([P, CH_D], f32, name="bq_sb", tag="bq_sb")
        nc.sync.dma_start(out=bq_sb[:], in_=tn["bq"].rearrange("(k p) -> p k", p=P))
        bf1_sb = constp.tile([P, CH_F], f32, name="bf1_sb", tag="bf1_sb")
        nc.sync.dma_start(out=bf1_sb[:], in_=tn["bf1p"].rearrange("(k p) -> p k", p=P))
        bke_sb = constp.tile([P, 2 * CH_D], bf16, name="bke_sb", tag="bke_sb")
        nc.sync.dma_start(out=bke_sb[:], in_=tn["bke2"][:, :])
        bvbc = constp.tile([P, D], bf16, name="bvbc", tag="bvbc")
        nc.sync.dma_start(out=bvbc[:], in_=tn["bv_bc"][:, :])
        bmbc = constp.tile([P, D], bf16, name="bmbc", tag="bmbc")
        nc.sync.dma_start(out=bmbc[:], in_=tn["bmlp_bc"][:, :])
        bf2bc = constp.tile([P, D], bf16, name="bf2bc", tag="bf2bc")
        nc.sync.dma_start(out=bf2bc[:], in_=tn["bf2_bc"][:, :])

        def _load_w(dram, tag):
            ws = []
            for ki in range(CH_D):
                t = wp.tile([P, D], bf16, name=f"{tag}{ki}", tag="wp")
                nc.sync.dma_start(out=t[:], in_=dram[ki * P:(ki + 1) * P, :])
                ws.append(t)
            return ws

        def _one_pass():
            for bi in range(BPC):
                # ---- Phase A: load x,h; cast bf16; transpose into hf_T [d,w] (h|x) ----
                hf = [hfp.tile([P, W], bf16, name=f"hf{dc}", tag="hf") for dc in range(CH_D)]
                for src, woff in ((hs, 0), (xs, C)):
                    natb = []
                    for ci in range(CH_C):
                        t = a1.tile([P, D], f32, name=f"nat{ci}", tag="a1")
                        nc.sync.dma_start(out=t[:], in_=src[bi, ci * P:(ci + 1) * P, :])
                        tb = natbp.tile([P, D], bf16, name=f"natb{ci}", tag="natb")
                        nc.scalar.copy(tb[:], t[:])
                        natb.append(tb)
                    for dc in range(CH_D):
                        ps = psp.tile([P, C], bf16, name="tpps", tag="ps")
                        for ci in range(CH_C):
                            nc.tensor.transpose(
                                ps[:, ci * P:(ci + 1) * P],
                                natb[ci][:, dc * P:(dc + 1) * P],
                                identb[:],
                            )
                        nc.scalar.copy(hf[dc][:, woff:woff + C], ps[:])

                # ---- Phase B: q_T = Wq x_T + bq ; qu_T = q_T + u_T ----
                wsb = _load_w(tn["wq_t"], "wq")
                qT, quT = [], []
                for do in range(CH_D):
                    ut = a5.tile([P, C], bf16, name=f"ut{do}", tag="a5")
                    nc.sync.dma_start(out=ut[:], in_=tn["u_t"][do * P:(do + 1) * P, :])
                    qps = psp.tile([P, C], f32, name="qps", tag="ps")
                    for ki in range(CH_D):
                        nc.tensor.matmul(
                            qps[:],
                            wsb[ki][:, do * P:(do + 1) * P],
                            hf[ki][:, C:W],
                            start=(ki == 0), stop=(ki == CH_D - 1),
                        )
                    q = a5.tile([P, C], bf16, name=f"qT{do}", tag="a5")
                    nc.scalar.add(q[:], qps[:], bq_sb[:, do:do + 1])
                    qu = a5.tile([P, C], bf16, name=f"quT{do}", tag="a5")
                    vtt(qu[:], q[:], ut[:], OP.add)
                    qT.append(q)
                    quT.append(qu)

                # ---- Phase T: t_T = Wke^T-fold: t = qu @ Wke  (t_T[j,c]) ----
                wsb = _load_w(tn["wke_n"], "wke")
                tT = []
                for jo in range(CH_D):
                    tps = psp.tile([P, C], f32, name="tps", tag="ps")
                    for ki in range(CH_D):
                        nc.tensor.matmul(
                            tps[:],
                            wsb[ki][:, jo * P:(jo + 1) * P],
                            quT[ki][:],
                            start=(ki == 0), stop=(ki == CH_D - 1),
                        )
                    t = a5.tile([P, C], bf16, name=f"tT{jo}", tag="a5")
                    nc.scalar.copy(t[:], tps[:])
                    tT.append(t)
                # rv[c] = qu . bke  (per-row additive constant for scores);
                # N=2 duplicated-column matmuls keep PSUM writes 8B-aligned
                rvps = psp.tile([P, C], f32, name="rvps", tag="ps")
                for ci in range(CH_C):
                    for ki in range(CH_D):
                        nc.tensor.matmul(
                            rvps[:, 2 * ci:2 * ci + 2],
                            quT[ki][:, ci * P:(ci + 1) * P],
                            bke_sb[:, 2 * ki:2 * ki + 2],
                            start=(ki == 0), stop=(ki == CH_D - 1),
                        )
                rv_sb = stp.tile([P, CH_C], f32, name="rv_sb", tag="rv")
                nc.scalar.copy(rv_sb[:], rvps[:, 0:2 * CH_C:2])

                # ---- Phase E: b = q @ kr^T -> DRAM scratch (natural [c,w]) ----
                krsb = _load_w(tn["kr_t"], "kr")
                pd = pdram.tile([C * W], f32, name="pd", tag="pd")
                pd2 = pd.rearrange("(c w) -> c w", w=W)
                for ci in range(CH_C):
                    for hh in range(2):
                        pps = psp.tile([P, C], f32, name="pps", tag="ps")
                        for ki in range(CH_D):
                            nc.tensor.matmul(
                                pps[:],
                                qT[ki][:, ci * P:(ci + 1) * P],
                                krsb[ki][:, hh * 512:(hh + 1) * 512],
                                start=(ki == 0), stop=(ki == CH_D - 1),
                            )
                        psb = psb5.tile([P, C], f32, name="psb", tag="psb")
                        nc.scalar.copy(psb[:], pps[:])
                        nc.sync.dma_start(
                            out=pd2[ci * P:(ci + 1) * P, hh * 512:(hh + 1) * 512],
                            in_=psb[:],
                        )

                # ---- Phase S: scores + softmax (masked tail chunks skipped) ----
                attb = []
                for ci in range(CH_C):
                    n1 = P * (ci + 1)        # live cols in hh=1 half
                    ah = []
                    for hh in range(2):
                        nn_ = C if hh == 0 else n1
                        aps = psp.tile([P, C], f32, name="aps", tag="ps")
                        for ki in range(CH_D):
                            nc.tensor.matmul(
                                aps[:, 0:nn_],
                                tT[ki][:, ci * P:(ci + 1) * P],
                                hf[ki][:, hh * 512:hh * 512 + nn_],
                                start=(ki == 0), stop=(ki == CH_D - 1),
                            )
                        ah.append(aps)
                    pskew = a1.tile([P, W], f32, name="pskew", tag="a1")
                    skew_ap = bass.AP(
                        tensor=pd.tensor,
                        offset=pd.offset + (W - 1) * P * ci + C - 1,
                        ap=[[W - 1, P], [1, W]],
                    )
                    nc.sync.dma_start(out=pskew[:], in_=skew_ap)
                    tf = a1.tile([P, W], f32, name="tf", tag="a1")
                    vtt(tf[:, 0:C], ah[0][:], pskew[:, 0:C], OP.add)
                    vtt(tf[:, C:C + n1], ah[1][:, 0:n1], pskew[:, C:C + n1], OP.add)
                    if n1 < C:
                        nc.vector.memset(tf[:, C + n1:W], 0.0)
                    ms = eattp.tile([P, W], bf16, name="ms", tag="eatt")
                    nc.sync.dma_start(out=ms[:], in_=tn["ms_b"][ci * P:(ci + 1) * P, :])
                    vk = eattp.tile([P, W], bf16, name="vk", tag="eatt")
                    nc.sync.dma_start(out=vk[:], in_=tn["vkr_ms_b"][ci * P:(ci + 1) * P, :])
                    # (tf + rv) * ms + vk ; masked cols stay 0 -> exp=1
                    nc.vector.tensor_scalar_add(tf[:], tf[:], rv_sb[:, ci:ci + 1])
                    vtt(tf[:], tf[:], ms[:], OP.mult)
                    vtt(tf[:], tf[:], vk[:], OP.add)
                    e = eattp.tile([P, W], bf16, name=f"e{ci}", tag="eatt")
                    zrow = stp.tile([P, 1], f32, name="zrow", tag="st")
                    nc.scalar.activation(e[:], tf[:], AF.Exp, accum_out=zrow[:])
                    rz = stp.tile([P, 1], f32, name="rz", tag="st")
                    nc.vector.reciprocal(rz[:], zrow[:])
                    ab = eattp.tile([P, W], bf16, name=f"att{ci}", tag="eatt")
                    nc.scalar.activation(ab[:], e[:], AF.Copy, bias=0.0, scale=rz[:])
                    attb.append(ab)

                # ---- Phase D: val = hf @ Wv^T + bv  (natural [w, d] layout) ----
                wsb = _load_w(tn["wv_t"], "wv")
                val = []
                for wc in range(CH_W):
                    vt = vp.tile([P, D], bf16, name=f"val{wc}", tag="val")
                    for hh in range(2):
                        vps = psp.tile([P, C], f32, name="vps", tag="ps")
                        for ki in range(CH_D):
                            nc.tensor.matmul(
                                vps[:],
                                hf[ki][:, wc * P:(wc + 1) * P],
                                wsb[ki][:, hh * 512:(hh + 1) * 512],
                                start=(ki == 0), stop=(ki == CH_D - 1),
                            )
                        vtt(vt[:, hh * 512:(hh + 1) * 512], vps[:],
                            bvbc[:, hh * 512:(hh + 1) * 512], OP.add)
                    val.append(vt)

                # ---- attT: transpose att (bf16) after D so PE doesn't stall ----
                attT = []
                for wc in range(CH_W):
                    tp = psp.tile([P, C], bf16, name="tp2", tag="ps")
                    for ci in range(CH_C):
                        nc.tensor.transpose(
                            tp[:, ci * P:(ci + 1) * P],
                            attb[ci][:, wc * P:(wc + 1) * P],
                            identb[:],
                        )
                    at = a5.tile([P, C], bf16, name=f"attT{wc}", tag="a5")
                    nc.scalar.copy(at[:], tp[:])
                    attT.append(at)

                # ---- Phase G: o_T = val^T @ att^T  [d, c] ----
                oT = []
                for do in range(CH_D):
                    ops = psp.tile([P, C], f32, name="ops", tag="ps")
                    for wc in range(CH_W):
                        nc.tensor.matmul(
                            ops[:],
                            val[wc][:, do * P:(do + 1) * P],
                            attT[wc][:],
                            start=(wc == 0), stop=(wc == CH_W - 1),
                        )
                    ot = a5.tile([P, C], bf16, name=f"oT{do}", tag="a5")
                    nc.scalar.copy(ot[:], ops[:])
                    oT.append(ot)

                # ---- Phase H: o2 = o @ Wmlp^T + bmlp ; LN -> z ; transpose z_T ----
                wsb = _load_w(tn["wmlp_t"], "wm")
                zb = []
                for ci in range(CH_C):
                    o2 = a1.tile([P, D], f32, name="o2", tag="a1")
                    s, ss = [], []
                    for hh in range(2):
                        o2ps = psp.tile([P, C], f32, name="o2ps", tag="ps")
                        for ki in range(CH_D):
                            nc.tensor.matmul(
                                o2ps[:],
                                oT[ki][:, ci * P:(ci + 1) * P],
                                wsb[ki][:, hh * 512:(hh + 1) * 512],
                                start=(ki == 0), stop=(ki == CH_D - 1),
                            )
                        vtt(o2[:, hh * 512:(hh + 1) * 512], o2ps[:],
                            bmbc[:, hh * 512:(hh + 1) * 512], OP.add)
                        sh = stp.tile([P, 1], f32, name="sh", tag="st")
                        sq = psb5.tile([P, C], f32, name="sq", tag="psb")
                        nc.scalar.activation(
                            sq[:], o2[:, hh * 512:(hh + 1) * 512], AF.Identity,
                            bias=0.0, accum_out=sh[:],
                        )
                        ssh = stp.tile([P, 1], f32, name="ssh", tag="st")
                        nc.scalar.activation(
                            sq[:], o2[:, hh * 512:(hh + 1) * 512], AF.Square,
                            bias=0.0, accum_out=ssh[:],
                        )
                        s.append(sh)
                        ss.append(ssh)
                    mu = stp.tile([P, 1], f32, name="mu", tag="st")
                    vtt(mu[:], s[0][:], s[1][:], OP.add)
                    nc.vector.tensor_scalar_mul(mu[:], mu[:], 1.0 / D)
                    ex2 = stp.tile([P, 1], f32, name="ex2", tag="st")
                    vtt(ex2[:], ss[0][:], ss[1][:], OP.add)
                    nc.vector.tensor_scalar_mul(ex2[:], ex2[:], 1.0 / D)
                    var = stp.tile([P, 1], f32, name="var", tag="st")
                    vtt(var[:], mu[:], mu[:], OP.mult)
                    vtt(var[:], ex2[:], var[:], OP.subtract)
                    nc.vector.tensor_scalar_add(var[:], var[:], EPS)
                    sd = stp.tile([P, 1], f32, name="sd", tag="st")
                    nc.scalar.activation(sd[:], var[:], AF.Sqrt, bias=0.0)
                    rstd = stp.tile([P, 1], f32, name="rstd", tag="st")
                    nc.vector.reciprocal(rstd[:], sd[:])
                    zc = eattp.tile([P, D], bf16, name="z", tag="eatt")
                    nc.vector.tensor_scalar(
                        out=zc[:], in0=o2[:], scalar1=mu[:], scalar2=rstd[:],
                        op0=OP.subtract, op1=OP.mult,
                    )
                    zb.append(zc)
                zT = []
                for dc in range(CH_D):
                    tp = psp.tile([P, C], bf16, name="tp3", tag="ps")
                    for ci in range(CH_C):
                        nc.tensor.transpose(
                            tp[:, ci * P:(ci + 1) * P],
                            zb[ci][:, dc * P:(dc + 1) * P],
                            identb[:],
                        )
                    zt = a5.tile([P, C], bf16, name=f"zT{dc}", tag="a5")
                    nc.scalar.copy(zt[:], tp[:])
                    zT.append(zt)

                # ---- Phase I: f_T = relu(Wf1g z_T + bf1')  (bf16 out) ----
                fT = []
                for jc in range(CH_F):
                    w1 = w1p.tile([P, CH_D, P], bf16, name="w1", tag="w1")
                    nc.sync.dma_start(
                        out=w1[:],
                        in_=tn["wf1_t"][:, jc * P:(jc + 1) * P].rearrange(
                            "(dc p) j -> p dc j", p=P
                        ),
                    )
                    fps = psp.tile([P, C], f32, name="fps", tag="ps")
                    for dc in range(CH_D):
                        nc.tensor.matmul(
                            fps[:], w1[:, dc, :], zT[dc][:],
                            start=(dc == 0), stop=(dc == CH_D - 1),
                        )
                    ft = ftp.tile([P, C], bf16, name=f"fT{jc}", tag="ft")
                    nc.scalar.activation(ft[:], fps[:], AF.Relu, bias=bf1_sb[:, jc:jc + 1])
                    fT.append(ft)

                # ---- Phase J: out = f @ Wf2^T + bf2 ----
                outps = [
                    [psp.tile([P, C], f32, name=f"op{ci}{hh}", tag="ps") for hh in range(2)]
                    for ci in range(CH_C)
                ]
                for jc in range(CH_F):
                    w2 = w2p.tile([P, D], bf16, name="w2", tag="w2")
                    nc.sync.dma_start(out=w2[:], in_=tn["wf2_b"][jc * P:(jc + 1) * P, :])
                    for ci in range(CH_C):
                        for hh in range(2):
                            nc.tensor.matmul(
                                outps[ci][hh][:],
                                fT[jc][:, ci * P:(ci + 1) * P],
                                w2[:, hh * 512:(hh + 1) * 512],
                                start=(jc == 0), stop=(jc == CH_F - 1),
                            )
                for ci in range(CH_C):
                    ob = a1.tile([P, D], f32, name="ob", tag="a1")
                    for hh in range(2):
                        vtt(ob[:, hh * 512:(hh + 1) * 512], outps[ci][hh][:],
                            bf2bc[:, hh * 512:(hh + 1) * 512], OP.add)
                    nc.sync.dma_start(out=outs[bi, ci * P:(ci + 1) * P, :], in_=ob[:])

        LOOP_R = int(os.environ.get("KERNEL_LOOP", "0"))
        if LOOP_R > 1:
            with tc.For_i(0, LOOP_R, 1):
                _one_pass()
        else:
            _one_pass()


def _build():
    if "nc" in _cached:
        return _cached["nc"]
    import concourse.mybir as mybir
    import concourse.tile as tile
    from concourse import bacc

    f32 = mybir.dt.float32
    bf16 = mybir.dt.bfloat16
    nc = bacc.Bacc("TRN2", target_bir_lowering=False, debug=False,
                   num_devices=NCORES)
    tn = {}
    tn["x"] = nc.dram_tensor("x", [BPC, C, D], f32, kind="ExternalInput")
    tn["h"] = nc.dram_tensor("h", [BPC, MEM, D], f32, kind="ExternalInput")
    for nm, shp in [
        ("wq_t", [D, D]), ("wke_n", [D, D]), ("wv_t", [D, D]), ("wmlp_t", [D, D]),
        ("kr_t", [D, W]), ("u_t", [D, C]), ("wf1_t", [D, FF]), ("wf2_b", [FF, D]),
        ("ms_b", [C, W]), ("vkr_ms_b", [C, W]), ("bke2", [P, 2 * CH_D]),
        ("bv_bc", [P, D]), ("bmlp_bc", [P, D]), ("bf2_bc", [P, D]),
    ]:
        tn[nm] = nc.dram_tensor(nm, shp, bf16, kind="ExternalInput")
    for nm, shp in [("bq", [D]), ("bf1p", [FF])]:
        tn[nm] = nc.dram_tensor(nm, shp, f32, kind="ExternalInput")
    tn["out"] = nc.dram_tensor("out", [BPC, C, D], f32, kind="ExternalOutput")

    with tile.TileContext(nc) as tc:
        _emit(nc, tc, tn)
    nc.compile()
    _cached["nc"] = nc
    return nc


def _host_consts(inputs):
    f = np.float32
    bf = ml_dtypes.bfloat16
    Wq, bq = inputs["Wq"].astype(f), inputs["bq"].astype(f)
    Wke, bke = inputs["Wke"].astype(f), inputs["bke"].astype(f)
    Wkr, bkr = inputs["Wkr"].astype(f), inputs["bkr"].astype(f)
    Wv, bv = inputs["Wv"].astype(f), inputs["bv"].astype(f)
    Wmlp, bmlp = inputs["Wmlp"].astype(f), inputs["bmlp"].astype(f)
    gamma, beta = inputs["gamma"].astype(f), inputs["beta"].astype(f)
    Wf1, bf1 = inputs["Wf1"].astype(f), inputs["bf1"].astype(f)
    Wf2, bf2 = inputs["Wf2"].astype(f), inputs["bf2"].astype(f)
    u, v, rr = inputs["u"].astype(f), inputs["v"].astype(f), inputs["r"].astype(f)

    kr = rr @ Wkr.T + bkr                      # [W, D]
    vkr = v @ kr.T                             # [C, W]
    mask = (np.arange(W)[None, :] <= np.arange(C)[:, None] + MEM)
    maskscale = (mask * ISQ).astype(f)
    cn = {
        "wq_t": np.ascontiguousarray(Wq.T).astype(bf),
        "wke_n": np.ascontiguousarray(Wke).astype(bf),
        "wv_t": np.ascontiguousarray(Wv.T).astype(bf),
        "wmlp_t": np.ascontiguousarray(Wmlp.T).astype(bf),
        "kr_t": np.ascontiguousarray(kr.T).astype(bf),
        "u_t": np.ascontiguousarray(u.T).astype(bf),
        "wf1_t": np.ascontiguousarray((Wf1 * gamma[None, :]).T).astype(bf),
        "wf2_b": np.ascontiguousarray(Wf2.T).astype(bf),
        "ms_b": maskscale.astype(bf),
        "vkr_ms_b": (vkr * maskscale).astype(bf),
        "bke2": np.ascontiguousarray(
            np.repeat(bke.reshape(CH_D, P).T, 2, axis=1)).astype(bf),
        "bv_bc": np.ascontiguousarray(np.broadcast_to(bv[None, :], (P, D))).astype(bf),
        "bmlp_bc": np.ascontiguousarray(np.broadcast_to(bmlp[None, :], (P, D))).astype(bf),
        "bf2_bc": np.ascontiguousarray(np.broadcast_to(bf2[None, :], (P, D))).astype(bf),
        "bq": bq,
        "bf1p": (bf1 + Wf1 @ beta).astype(f),
    }
    return cn


def kernel(**inputs):
    from concourse.bass_utils import run_bass_kernel_spmd

    nc = _build()
    cn = _host_consts(inputs)
    x = np.ascontiguousarray(inputs["x"].astype(np.float32))
    h = np.ascontiguousarray(inputs["h"].astype(np.float32))
    in_maps = []
    for i in range(NCORES):
        m = dict(cn)
        m["x"] = np.ascontiguousarray(x[i * BPC:(i + 1) * BPC])
        m["h"] = np.ascontiguousarray(h[i * BPC:(i + 1) * BPC])
        in_maps.append(m)
    res = run_bass_kernel_spmd(nc, in_maps, list(range(NCORES)))
    out = np.concatenate([res.results[i]["out"] for i in range(NCORES)], axis=0)
    return out.astype(np.float32)

